# revision 22
# baseline (speedup 1.0000x reference)
"""Trainium2 Bass kernel for nn_DeepConv1d (self-contained).

Math (per batch b):
  xr   = linear-interp(deep, 1024 -> 4096)           # commutes with 1x1 conv
  y    = conv_w @ xr + conv_b                        # == interp(conv_w @ deep + conv_b)
  xs   = GAMA*(y-mean)/(var_unbiased+EPS)            # per-channel over n
  loss_k[c,l] = sech^2(f*(y_pad[c,l+k]-y_pad[c,l+3]))  # k=0..6, reflect pad 3
  S    = sum_k loss_k ;  W_k = (loss_k/S)*x_pad[:,l+k]
  out[o,l] = sum_{c,k} fc_w[o, 7c+k] * W_k[c,l]

On-chip identities / structure:
  - interp(conv(.)) == conv(interp(.)); conv+interp fused as 4 phase
    matmuls against host-stacked [dp; dp_shifted] (y bias dropped: it
    cancels in the y-differences, and mean/var are computed exactly on
    the host, so f = GAMA/(var+EPS) arrives as a per-channel constant).
  - loss = sech^2 = 1 - tanh^2: ACT Tanh (scale=f) then ACT Square give
    t2_g = tanh^2 per gap g=|k-3| in {1,2,3}; a DVE tensor_scalar
    (4x mode) forms loss_g = 1 - t2_g for the W products.
  - S-sum runs on the PE from the t2 arrays directly: PSUM =
    7 - sum(t2 terms) via 6 accumulating matmuls with lhsT = -I plus a
    +7 constant pass, so no extra DVE work on the S path.
  - G = 1/S via DVE reciprocal_approx_fast (fp32) + ACT cast to bf16.
  - Final GEMM (7 taps, contract 64 per batch) interleaves the two
    batches k-by-k: their lhsT/rhs live on disjoint partition halves so
    the PE runs them concurrently on separate row-groups/PSUM banks.

Layout: 2 batches per core packed on 128 partitions (64 channels each).
"""
import contextlib

import numpy as np
import ml_dtypes

import concourse.bass as bass
import concourse.bacc as bacc_mod
import concourse.mybir as mybir
import concourse.tile as tile
from concourse.bass_utils import run_bass_kernel_spmd

bf16 = ml_dtypes.bfloat16
AF = mybir.ActivationFunctionType
ALU = mybir.AluOpType

KS = 7
PAD = 3
GAMA = 0.5
EPS = 1e-9
N = 4096
ND = 1024
NP = N + 2 * PAD       # 4102
L3 = N + PAD           # 4099: gap-array length
NCORES = 8
NCH = 4                # l-chunks
CW = N // NCH          # 1024

F32 = mybir.dt.float32
BF = mybir.dt.bfloat16

HH = 2052              # A-half width for dy/tanh/square slices
H0 = 1032              # first slice: covers chunk-0 S-sum + GL reads
NWARM = 46             # PE keep-warm dummy matmuls between conv and S-sums


def kernel_body(tc, blobA_d, fv_d, blobB_d, out_d):
    nc = tc.nc

    ctx = contextlib.ExitStack()
    with ctx:
        io = ctx.enter_context(tc.tile_pool(name="io", bufs=1))
        mid = ctx.enter_context(tc.tile_pool(name="mid", bufs=1))
        loss = ctx.enter_context(tc.tile_pool(name="loss", bufs=1))
        ck = ctx.enter_context(tc.tile_pool(name="ck", bufs=2))
        stp = ctx.enter_context(tc.tile_pool(name="stp", bufs=3))
        pp = ctx.enter_context(tc.tile_pool(name="pp", bufs=2, space="PSUM"))
        pm = ctx.enter_context(tc.tile_pool(name="pm", bufs=1, space="PSUM"))
        ppa = ctx.enter_context(tc.tile_pool(name="ppa", bufs=4, space="PSUM"))

        # ------------- input DMAs (conv-critical, small, first) --------
        # cph/dpq stay separate transfers: the first conv LDW/matmul waits
        # on their individual completion semaphores, so a merged blob
        # would delay conv to the blob's full arrival.
        cph = io.tile([64, 512], BF, tag="cph")      # 4 phases x 128, flat
        nc.sync.dma_start(out=cph, in_=blobA_d[:, 0:512])
        dpq01 = io.tile([64, ND], BF, tag="dpq01")
        nc.sync.dma_start(out=dpq01, in_=blobA_d[:, 512:512 + ND])
        dpq23 = io.tile([64, ND], BF, tag="dpq23")
        nc.sync.dma_start(out=dpq23, in_=blobA_d[:, 512 + ND:])
        fv = io.tile([128, 1], F32, tag="fv")
        nc.sync.dma_start(out=fv, in_=fv_d[:, :])
        # blobB [128, 128+896+4102]: nid | fck (7x128) | xp
        blobB = io.tile([128, 128 + KS * 128 + NP], BF, tag="blobB")
        nc.sync.dma_start(out=blobB, in_=blobB_d[:, :])
        nid = blobB[:, 0:128]
        fckf = blobB[:, 128:128 + KS * 128]          # [128, 7*128] flat
        xp = blobB[:, 128 + KS * 128:]               # [128, NP] reflect-padded x

        # warm the tanh table with a DMA-independent input
        wz = mid.tile([128, 1], F32, tag="wz")
        nc.vector.memset(wz, 0.0)
        warm = mid.tile([128, 1], F32, tag="warm")
        nc.scalar.activation(out=warm, in_=wz, func=AF.Tanh, scale=1.0)

        # (-I) @ (-7) = +7 per partition: same nid lhsT as the t2 terms,
        # so the S-sum accumulation never switches weights.
        q512 = io.tile([128, 512], BF, tag="q512")
        nc.vector.memset(q512, -7.0)

        # ------- conv+interp fused on the PE (phase-decomposed) ----------
        # y[4j+r] = a_r*ys[j] + b_r*ys[j+s_r]  (s=-1 for r<2, +1 for r>=2)
        # == one matmul per phase against host-stacked [dp; dp_shifted].
        # h-major order through a ring of 2 one-bank PSUM tiles, so the
        # DVE can interleave the four h0 halves (plus a 1-col touch-up of
        # each h1 tile) and start dy after a few short casts.
        ypad = mid.tile([128, NP], BF, tag="ypad")
        yph = {}
        for h in range(2):
            for r in range(4):
                ypr = pp.tile([128, 512], F32, tag="ys", name=f"yph{r}_{h}")
                dq = dpq01 if r < 2 else dpq23
                nc.tensor.matmul(
                    out=ypr,
                    lhsT=cph[:, r * 128:(r + 1) * 128],
                    rhs=dq[:, h * 512:(h + 1) * 512],
                    start=True, stop=True,
                )
                yph[(r, h)] = ypr

        # PE keep-warm: dummy matmuls into a scratch bank (output never
        # read). rhs slices of xp make them eligible only once the big
        # input DMA lands (~15us), so the scheduler cannot hoist them
        # before conv; back-to-back they bridge the PE-idle dy/tanh
        # window so the HAM stays at K=8/8 for the S-sums and GEMMs.
        scr = ppa.tile([128, 512], F32, tag="acc", name="warmmm")
        for i in range(NWARM):
            nc.tensor.matmul(out=scr, lhsT=q512[:, 0:128],
                             rhs=xp[:, (i % 7) * 512:(i % 7) * 512 + 512],
                             start=True, stop=True, skip_group_check=True)

        def interleave(r, h, j0, jw):
            dst = bass.AP(tensor=ypad.tensor,
                          offset=ypad.offset + PAD + r + 4 * (h * 512 + j0),
                          ap=[list(ypad.ap[0]), [4, jw]])
            nc.vector.tensor_copy(out=dst, in_=yph[(r, h)][:, j0:j0 + jw])

        # slice 0 casts: ypad[:, :1035) -> dy slice 0 -> tanh starts early
        JS = (H0 + 3 + 3) // 4  # 259: j-cols per phase covering ypad[0,1035)
        for r in range(4):
            interleave(r, 0, 0, JS)
        # reflect front edges: ypad[2-i] = ypad[4+i]
        for i in range(3):
            nc.vector.tensor_copy(out=ypad[:, 2 - i:3 - i],
                                  in_=ypad[:, 4 + i:5 + i])

        dy1 = loss.tile([128, L3], BF, tag="T1")
        dy2b = loss.tile([128, L3], BF, tag="T2")
        dy3 = loss.tile([128, L3], BF, tag="T3")
        SL0, SL1 = slice(0, H0), slice(H0, HH)
        SL_B = slice(HH, L3)

        def dy_emit(sl):
            lo, hi = sl.start, sl.stop
            w = hi - lo
            nc.vector.tensor_sub(out=dy3[:, sl], in0=ypad[:, lo + 3:lo + 3 + w],
                                 in1=ypad[:, lo:hi])
            nc.vector.tensor_sub(out=dy2b[:, sl], in0=ypad[:, lo + 3:lo + 3 + w],
                                 in1=ypad[:, lo + 1:lo + 1 + w])
            nc.vector.tensor_sub(out=dy1[:, sl], in0=ypad[:, lo + 1:lo + 1 + w],
                                 in1=ypad[:, lo:hi])

        dy_emit(SL0)
        # rest of h0 + the h1 touch-up col -> dy slice 1
        for r in range(4):
            interleave(r, 0, JS, 512 - JS)
        for r in range(4):
            interleave(r, 1, 0, 1)   # col 2051+r: completes ypad[:, :2055)
        dy_emit(SL1)
        # B-half interleaves + tail edges + B-half dys
        for r in range(4):
            interleave(r, 1, 1, 511)
        for i in range(3):
            nc.vector.tensor_copy(out=ypad[:, N + 3 + i:N + 4 + i],
                                  in_=ypad[:, N + 1 - i:N + 2 - i])
        dy_emit(SL_B)

        # ------------- loss: t = tanh(f*dy); t2 = t^2 (ACT) -------------
        # t is a single scratch tile (all producers/consumers on ACT, so
        # reuse costs no cross-engine sync); t2 arrays feed the PE S-sum
        # and the DVE negates. Sliced so the chunk-0 S-sum and muls can
        # start after the first third of the chain.
        tsc = loss.tile([128, L3], BF, tag="TS")
        t2_3 = loss.tile([128, L3], BF, tag="Q3")
        t2_2 = loss.tile([128, L3], BF, tag="Q2")
        t2_1 = loss.tile([128, L3], BF, tag="Q1")
        gaps = ((dy3, t2_3), (dy2b, t2_2), (dy1, t2_1))
        for sl in (SL0, SL1, SL_B):
            for dy, t2 in gaps:
                nc.scalar.activation(out=tsc[:, sl], in_=dy[:, sl],
                                     func=AF.Tanh, scale=fv)
                nc.scalar.activation(out=t2[:, sl], in_=tsc[:, sl],
                                     func=AF.Square)

        # loss_g = 1 - t2_g (DVE tensor_scalar, 4x mode); gates only the
        # per-chunk GL/P products, not the S-sum.
        ls3 = loss.tile([128, L3], BF, tag="L3")
        ls2 = loss.tile([128, L3], BF, tag="L2")
        ls1 = loss.tile([128, L3], BF, tag="L1")
        nls = ((t2_3, ls3), (t2_2, ls2), (t2_1, ls1))

        def neg_emit(sl):
            for t2, ls in nls:
                nc.vector.tensor_scalar(out=ls[:, sl], in0=t2[:, sl],
                                        scalar1=-1.0, scalar2=1.0,
                                        op0=ALU.mult, op1=ALU.add)

        neg_emit(SL0)

        # S-sum terms: PSUM = 7 - sum(t2 terms) accumulated on the PE
        terms = [(t2_1, 2), (t2_1, 3), (t2_2, 0), (t2_2, 2), (t2_3, 0), (t2_3, 3)]
        W_of = {}

        def emit_front(c):
            """msum (PE) -> G (DVE recip + ACT cast) -> P/GL/W (DVE).
            The recip is emitted before the muls: msum is a ring of one,
            so the next chunk's S-sum waits on it."""
            lo = c * CW
            msum_ps = pm.tile([128, CW], F32, tag="ms", name=f"msum{c}")
            for h in range(2):
                base = lo + h * 512
                sub = msum_ps[:, h * 512:(h + 1) * 512]
                nc.tensor.matmul(out=sub, lhsT=nid, rhs=q512,
                                 start=True, stop=False)
                for t, (arr, off) in enumerate(terms):
                    nc.tensor.matmul(
                        out=sub, lhsT=nid,
                        rhs=arr[:, base + off:base + off + 512],
                        start=False, stop=(t == 5),
                    )
            # G cast runs on the DVE: on ACT it would queue behind the
            # whole loss chain and stall every chunk's W muls by ~10us.
            G = ck.tile([128, CW], BF, tag="G4", name=f"G_{c}")
            G32 = ck.tile([128, CW], F32, tag="G32", name=f"G32_{c}")
            nc.vector.reciprocal_approx_fast(out=G32, in_=msum_ps)
            nc.vector.tensor_copy(out=G, in_=G32)

            Pc0 = ck.tile([128, CW], BF, tag="P0", name=f"P0_{c}")
            Pc1 = ck.tile([128, CW], BF, tag="P1", name=f"P1_{c}")
            Pc2 = ck.tile([128, CW], BF, tag="P2", name=f"P2_{c}")
            nc.vector.tensor_mul(out=Pc0, in0=ls3[:, lo:lo + CW],
                                 in1=xp[:, lo:lo + CW])
            nc.vector.tensor_mul(out=Pc1, in0=ls2[:, lo:lo + CW],
                                 in1=xp[:, lo + 1:lo + 1 + CW])
            nc.vector.tensor_mul(out=Pc2, in0=ls1[:, lo + 2:lo + 2 + CW],
                                 in1=xp[:, lo + 2:lo + 2 + CW])

            GL1 = ck.tile([128, CW], BF, tag="GL1", name=f"GL1_{c}")
            GL2 = ck.tile([128, CW], BF, tag="GL2", name=f"GL2_{c}")
            GL3 = ck.tile([128, CW], BF, tag="GL3", name=f"GL3_{c}")
            nc.vector.tensor_mul(out=GL1, in0=ls1[:, lo + 3:lo + 3 + CW], in1=G)
            nc.vector.tensor_mul(out=GL2, in0=ls2[:, lo + 2:lo + 2 + CW], in1=G)
            nc.vector.tensor_mul(out=GL3, in0=ls3[:, lo + 3:lo + 3 + CW], in1=G)

            W = [ck.tile([128, CW], BF, tag=f"W{k}", name=f"W{k}_{c}")
                 for k in range(KS)]
            nc.vector.tensor_mul(out=W[0], in0=G, in1=Pc0)
            nc.vector.tensor_mul(out=W[1], in0=G, in1=Pc1)
            nc.vector.tensor_mul(out=W[2], in0=G, in1=Pc2)
            nc.vector.tensor_mul(out=W[3], in0=G, in1=xp[:, lo + 3:lo + 3 + CW])
            nc.vector.tensor_mul(out=W[4], in0=GL1, in1=xp[:, lo + 4:lo + 4 + CW])
            nc.vector.tensor_mul(out=W[5], in0=GL2, in1=xp[:, lo + 5:lo + 5 + CW])
            nc.vector.tensor_mul(out=W[6], in0=GL3, in1=xp[:, lo + 6:lo + 6 + CW])
            W_of[c] = W

        def emit_back(c):
            """GEMM (PE, b0/b1 interleaved k-by-k so the two batches run on
            disjoint row-groups concurrently) -> obuf copies -> DMA."""
            lo = c * CW
            W = W_of[c]
            obuf = stp.tile([128, 2, CW], BF, tag="obuf", name=f"obuf_{c}")
            for sub_i in range(CW // 512):
                cs = slice(sub_i * 512, (sub_i + 1) * 512)
                acc = [ppa.tile([128, 512], F32, tag="acc",
                                name=f"acc_{c}_{b}_{sub_i}") for b in range(2)]
                for k in range(KS):
                    for b in range(2):
                        prow = slice(64 * b, 64 * (b + 1))
                        nc.tensor.matmul(
                            out=acc[b][:, :],
                            lhsT=fckf[prow, k * 128:(k + 1) * 128],
                            rhs=W[k][prow, cs],
                            start=(k == 0), stop=(k == KS - 1),
                        )
                for b in range(2):
                    # last chunk: split copies DVE/ACT so the tail runs
                    # them in parallel (DVE is already drained by then)
                    if c == NCH - 1 and sub_i == 1:
                        nc.vector.tensor_copy(out=obuf[:, b, cs], in_=acc[b])
                    else:
                        nc.scalar.copy(out=obuf[:, b, cs], in_=acc[b])
                if c == NCH - 1:
                    # last chunk: DMA each 512-sub as it lands
                    nc.sync.dma_start(out=out_d[:, c, :, cs],
                                      in_=obuf[:, :, cs])
            if c != NCH - 1:
                nc.sync.dma_start(out=out_d[:, c, :, :], in_=obuf)

        # Emission order: chunk 0 is covered by slice 0, chunk 1 by
        # slices 0-1, chunks 2/3 by the B slice; negates for each slice
        # land just before the first front that needs them.
        emit_front(0)
        neg_emit(SL1)
        emit_front(1)
        emit_back(0)
        neg_emit(SL_B)
        emit_back(1)
        emit_front(2)
        emit_back(2)
        emit_front(3)
        emit_back(3)


def build_nc():
    nc = bacc_mod.Bacc(None, target_bir_lowering=False)
    blobA_d = nc.dram_tensor("blobA", [64, 512 + 2 * ND], BF,
                             kind="ExternalInput")
    fv_d = nc.dram_tensor("fv", [128, 1], F32, kind="ExternalInput")
    blobB_d = nc.dram_tensor("blobB", [128, 128 + KS * 128 + NP], BF,
                             kind="ExternalInput")
    out_d = nc.dram_tensor("out", [128, NCH, 2, CW], BF, kind="ExternalOutput")
    with tile.TileContext(nc) as tc:
        kernel_body(tc, blobA_d, fv_d, blobB_d, out_d)
    nc.compile()
    return nc


def _interp_host(dp, n_out):
    # exact port of the reference's interp_linear on the last dim (fp64)
    n_in = dp.shape[-1]
    src = (np.arange(n_out, dtype=np.float64) + 0.5) * (n_in / n_out) - 0.5
    src = np.clip(src, 0.0, n_in - 1.0)
    lo = np.floor(src).astype(np.int64)
    hi = np.minimum(lo + 1, n_in - 1)
    w = src - lo
    return dp[..., lo] * (1.0 - w) + dp[..., hi] * w


def prep_inputs(deep, x, conv_w, conv_b, fc_w):
    deep = np.asarray(deep, np.float32)
    x = np.asarray(x, np.float32)
    conv_w = np.asarray(conv_w, np.float32)
    conv_b = np.asarray(conv_b, np.float32)
    fc_w = np.asarray(fc_w, np.float32)

    xpad = np.pad(x, ((0, 0), (0, 0), (PAD, PAD)), mode="reflect")
    xp_all = np.ascontiguousarray(xpad.reshape(NCORES, 128, NP)).astype(bf16)
    dp_all = np.ascontiguousarray(deep.reshape(NCORES, 32, ND))
    # phase-fused conv+interp weights: y[4j+r] = a_r*ys[j] + b_r*ys[j+s_r]
    a_ph = [0.625, 0.875, 0.875, 0.625]
    b_ph = [0.375, 0.125, 0.125, 0.375]
    cwT = conv_w.T  # (16, 64)
    cph = np.zeros((64, 4, 128), np.float32)
    for r in range(4):
        cph[0:16, r, 0:64] = a_ph[r] * cwT
        cph[16:32, r, 64:128] = a_ph[r] * cwT
        cph[32:48, r, 0:64] = b_ph[r] * cwT
        cph[48:64, r, 64:128] = b_ph[r] * cwT
    cph = np.ascontiguousarray(cph).astype(bf16)
    nid = (-np.eye(128)).astype(bf16)
    fc3 = fc_w.reshape(128, 64, KS)
    fck_half = np.transpose(fc3, (1, 2, 0)).copy()
    fck = np.ascontiguousarray(
        np.concatenate([fck_half, fck_half], axis=0)).astype(bf16)

    # exact per-channel variance of y on the host (fp64): f = GAMA/(var+EPS)
    xr = _interp_host(deep.astype(np.float64), N)          # (16, dc, N)
    s1 = xr.sum(axis=2)                                    # (16, dc)
    g2 = np.einsum('bdn,ben->bde', xr, xr)                 # (16, dc, dc)
    w64 = conv_w.astype(np.float64)
    cb64 = conv_b.astype(np.float64)
    sy = np.einsum('cd,bd->bc', w64, s1) + N * cb64[None, :]
    sy2 = (np.einsum('cd,bde,ce->bc', w64, g2, w64)
           + 2.0 * cb64[None, :] * np.einsum('cd,bd->bc', w64, s1)
           + N * cb64[None, :] ** 2)
    mean = sy / N
    var = (sy2 - N * mean ** 2) / (N - 1)
    f = (GAMA / (var + EPS)).astype(np.float32)            # (16, 64)
    f_all = f.reshape(NCORES, 128, 1)

    # blobA: cph flattened [64, 512] | dpq01 [64, 1024] | dpq23 [64, 1024]
    cph_flat = cph.reshape(64, 512)
    maps = []
    for ci in range(NCORES):
        dp2 = dp_all[ci]                                  # (32, ND) b0;b1
        dpm = np.concatenate([dp2[:, :1], dp2[:, :-1]], axis=1)   # dp[j-1]
        dpp = np.concatenate([dp2[:, 1:], dp2[:, -1:]], axis=1)   # dp[j+1]
        dpq01 = np.concatenate([dp2, dpm], axis=0).astype(bf16)   # [64, ND]
        dpq23 = np.concatenate([dp2, dpp], axis=0).astype(bf16)
        blobA = np.ascontiguousarray(
            np.concatenate([cph_flat, dpq01, dpq23], axis=1))
        blobB = np.ascontiguousarray(np.concatenate(
            [nid, fck.reshape(128, KS * 128), xp_all[ci]], axis=1))
        maps.append({"blobA": blobA, "fv": np.ascontiguousarray(f_all[ci]),
                     "blobB": blobB})
    return maps


def gather_out(results):
    out_full = np.empty((16, 128, N), np.float32)
    for ci in range(NCORES):
        o = np.asarray(results[ci]["out"], dtype=np.float32)
        o = np.transpose(o, (0, 2, 1, 3)).reshape(128, 2, N)
        out_full[2 * ci] = o[:, 0]
        out_full[2 * ci + 1] = o[:, 1]
    return out_full


_CACHED = {}


def _get_nc():
    if "nc" not in _CACHED:
        _CACHED["nc"] = build_nc()
    return _CACHED["nc"]


def kernel(deep, x, conv_w, conv_b, fc_w):
    in_maps = prep_inputs(deep, x, conv_w, conv_b, fc_w)
    nc = _get_nc()
    res = run_bass_kernel_spmd(nc, in_maps, core_ids=list(range(NCORES)))
    return gather_out(res.results)


# revision 27
# speedup vs baseline: 1.0011x; 1.0011x over previous
"""Trainium2 Bass kernel for nn_DeepConv1d (self-contained).

Math (per batch b):
  xr   = linear-interp(deep, 1024 -> 4096)           # commutes with 1x1 conv
  y    = conv_w @ xr + conv_b                        # == interp(conv_w @ deep + conv_b)
  xs   = GAMA*(y-mean)/(var_unbiased+EPS)            # per-channel over n
  loss_k[c,l] = sech^2(f*(y_pad[c,l+k]-y_pad[c,l+3]))  # k=0..6, reflect pad 3
  S    = sum_k loss_k ;  W_k = (loss_k/S)*x_pad[:,l+k]
  out[o,l] = sum_{c,k} fc_w[o, 7c+k] * W_k[c,l]

On-chip identities / structure:
  - interp(conv(.)) == conv(interp(.)); conv+interp fused as 4 phase
    matmuls against host-stacked [dp; dp_shifted] (y bias dropped: it
    cancels in the y-differences, and mean/var are computed exactly on
    the host, so f = GAMA/(var+EPS) arrives as a per-channel constant).
  - loss = sech^2 = 1 - tanh^2: ACT Tanh (scale=f) then ACT Square give
    t2_g = tanh^2 per gap g=|k-3| in {1,2,3}; a DVE tensor_scalar
    (4x mode) forms loss_g = 1 - t2_g for the W products.
  - S-sum runs on the PE from the t2 arrays directly: PSUM =
    7 - sum(t2 terms) via 6 accumulating matmuls with lhsT = -I plus a
    +7 constant pass, so no extra DVE work on the S path.
  - G = 1/S via DVE reciprocal_approx_fast (fp32) + ACT cast to bf16.
  - Final GEMM (7 taps, contract 64 per batch) interleaves the two
    batches k-by-k: their lhsT/rhs live on disjoint partition halves so
    the PE runs them concurrently on separate row-groups/PSUM banks.

Layout: 2 batches per core packed on 128 partitions (64 channels each).
"""
import contextlib

import numpy as np
import ml_dtypes

import concourse.bass as bass
import concourse.bacc as bacc_mod
import concourse.mybir as mybir
import concourse.tile as tile
from concourse.bass_utils import run_bass_kernel_spmd

bf16 = ml_dtypes.bfloat16
AF = mybir.ActivationFunctionType
ALU = mybir.AluOpType

KS = 7
PAD = 3
GAMA = 0.5
EPS = 1e-9
N = 4096
ND = 1024
NP = N + 2 * PAD       # 4102
L3 = N + PAD           # 4099: gap-array length
NCORES = 8
NCH = 4                # l-chunks
CW = N // NCH          # 1024

F32 = mybir.dt.float32
BF = mybir.dt.bfloat16

HH = 2052              # A-half width for dy/tanh/square slices
H0 = 1032              # first slice: covers chunk-0 S-sum + GL reads
NWARM = 46             # PE keep-warm dummy matmuls between conv and S-sums


def kernel_body(tc, blobA_d, fv_d, blobB_d, out_d):
    nc = tc.nc

    ctx = contextlib.ExitStack()
    with ctx:
        io = ctx.enter_context(tc.tile_pool(name="io", bufs=1))
        mid = ctx.enter_context(tc.tile_pool(name="mid", bufs=1))
        loss = ctx.enter_context(tc.tile_pool(name="loss", bufs=1))
        ck = ctx.enter_context(tc.tile_pool(name="ck", bufs=2))
        stp = ctx.enter_context(tc.tile_pool(name="stp", bufs=3))
        pp = ctx.enter_context(tc.tile_pool(name="pp", bufs=2, space="PSUM"))
        pm = ctx.enter_context(tc.tile_pool(name="pm", bufs=1, space="PSUM"))
        ppa = ctx.enter_context(tc.tile_pool(name="ppa", bufs=4, space="PSUM"))

        # ------------- input DMAs (conv-critical, small, first) --------
        # cph/dpq stay separate transfers: the first conv LDW/matmul waits
        # on their individual completion semaphores, so a merged blob
        # would delay conv to the blob's full arrival.
        cph = io.tile([64, 512], BF, tag="cph")      # 4 phases x 128, flat
        nc.sync.dma_start(out=cph, in_=blobA_d[:, 0:512])
        dpq01 = io.tile([64, ND], BF, tag="dpq01")
        nc.sync.dma_start(out=dpq01, in_=blobA_d[:, 512:512 + ND])
        dpq23 = io.tile([64, ND], BF, tag="dpq23")
        nc.sync.dma_start(out=dpq23, in_=blobA_d[:, 512 + ND:])
        fv = io.tile([128, 1], F32, tag="fv")
        nc.sync.dma_start(out=fv, in_=fv_d[:, :])
        # blobB [128, 128+896+4102]: nid | fck (7x128) | xp
        blobB = io.tile([128, 128 + KS * 128 + NP], BF, tag="blobB")
        nc.sync.dma_start(out=blobB, in_=blobB_d[:, :])
        nid = blobB[:, 0:128]
        fckf = blobB[:, 128:128 + KS * 128]          # [128, 7*128] flat
        xp = blobB[:, 128 + KS * 128:]               # [128, NP] reflect-padded x

        # warm the tanh table with a DMA-independent input
        wz = mid.tile([128, 1], F32, tag="wz")
        nc.vector.memset(wz, 0.0)
        warm = mid.tile([128, 1], F32, tag="warm")
        nc.scalar.activation(out=warm, in_=wz, func=AF.Tanh, scale=1.0)

        # (-I) @ (-7) = +7 per partition: same nid lhsT as the t2 terms,
        # so the S-sum accumulation never switches weights.
        q512 = io.tile([128, 512], BF, tag="q512")
        nc.vector.memset(q512, -7.0)

        # ------- conv+interp fused on the PE (phase-decomposed) ----------
        # y[4j+r] = a_r*ys[j] + b_r*ys[j+s_r]  (s=-1 for r<2, +1 for r>=2)
        # == one matmul per phase against host-stacked [dp; dp_shifted].
        # h-major order through a ring of 2 one-bank PSUM tiles, so the
        # DVE can interleave the four h0 halves (plus a 1-col touch-up of
        # each h1 tile) and start dy after a few short casts.
        ypad = mid.tile([128, NP], BF, tag="ypad")
        yph = {}
        for h in range(2):
            for r in range(4):
                ypr = pp.tile([128, 512], F32, tag="ys", name=f"yph{r}_{h}")
                dq = dpq01 if r < 2 else dpq23
                nc.tensor.matmul(
                    out=ypr,
                    lhsT=cph[:, r * 128:(r + 1) * 128],
                    rhs=dq[:, h * 512:(h + 1) * 512],
                    start=True, stop=True,
                )
                yph[(r, h)] = ypr

        # PE keep-warm: dummy matmuls into a scratch bank (output never
        # read). rhs slices of xp make them eligible only once the big
        # input DMA lands (~15us), so the scheduler cannot hoist them
        # before conv; back-to-back they bridge the PE-idle dy/tanh
        # window so the HAM stays at K=8/8 for the S-sums and GEMMs.
        scr = ppa.tile([128, 512], F32, tag="acc", name="warmmm")
        for i in range(NWARM):
            nc.tensor.matmul(out=scr, lhsT=q512[:, 0:128],
                             rhs=xp[:, (i % 7) * 512:(i % 7) * 512 + 512],
                             start=True, stop=True, skip_group_check=True)

        def interleave(r, h, j0, jw):
            dst = bass.AP(tensor=ypad.tensor,
                          offset=ypad.offset + PAD + r + 4 * (h * 512 + j0),
                          ap=[list(ypad.ap[0]), [4, jw]])
            nc.vector.tensor_copy(out=dst, in_=yph[(r, h)][:, j0:j0 + jw])

        # slice 0 casts: ypad[:, :1035) -> dy slice 0 -> tanh starts early
        JS = (H0 + 3 + 3) // 4  # 259: j-cols per phase covering ypad[0,1035)
        for r in range(4):
            interleave(r, 0, 0, JS)
        # reflect front edges: ypad[2-i] = ypad[4+i]
        for i in range(3):
            nc.vector.tensor_copy(out=ypad[:, 2 - i:3 - i],
                                  in_=ypad[:, 4 + i:5 + i])

        dy1 = loss.tile([128, L3], BF, tag="T1")
        dy2b = loss.tile([128, L3], BF, tag="T2")
        dy3 = loss.tile([128, L3], BF, tag="T3")
        SL0, SL1 = slice(0, H0), slice(H0, HH)
        SL_B = slice(HH, L3)

        def dy_emit(sl):
            lo, hi = sl.start, sl.stop
            w = hi - lo
            nc.vector.tensor_sub(out=dy3[:, sl], in0=ypad[:, lo + 3:lo + 3 + w],
                                 in1=ypad[:, lo:hi])
            nc.vector.tensor_sub(out=dy2b[:, sl], in0=ypad[:, lo + 3:lo + 3 + w],
                                 in1=ypad[:, lo + 1:lo + 1 + w])
            nc.vector.tensor_sub(out=dy1[:, sl], in0=ypad[:, lo + 1:lo + 1 + w],
                                 in1=ypad[:, lo:hi])

        dy_emit(SL0)
        # rest of h0 + the h1 touch-up col -> dy slice 1
        for r in range(4):
            interleave(r, 0, JS, 512 - JS)
        for r in range(4):
            interleave(r, 1, 0, 1)   # col 2051+r: completes ypad[:, :2055)
        dy_emit(SL1)
        # B-half interleaves + tail edges + B-half dys
        for r in range(4):
            interleave(r, 1, 1, 511)
        for i in range(3):
            nc.vector.tensor_copy(out=ypad[:, N + 3 + i:N + 4 + i],
                                  in_=ypad[:, N + 1 - i:N + 2 - i])
        dy_emit(SL_B)

        # ------------- loss: t = tanh(f*dy); t2 = t^2 (ACT) -------------
        # t is a single scratch tile (all producers/consumers on ACT, so
        # reuse costs no cross-engine sync); t2 arrays feed the PE S-sum
        # and the DVE negates. Sliced so the chunk-0 S-sum and muls can
        # start after the first third of the chain.
        tsc = loss.tile([128, L3], BF, tag="TS")
        t2_3 = loss.tile([128, L3], BF, tag="Q3")
        t2_2 = loss.tile([128, L3], BF, tag="Q2")
        t2_1 = loss.tile([128, L3], BF, tag="Q1")
        gaps = ((dy3, t2_3), (dy2b, t2_2), (dy1, t2_1))
        for sl in (SL0, SL1, SL_B):
            for dy, t2 in gaps:
                nc.scalar.activation(out=tsc[:, sl], in_=dy[:, sl],
                                     func=AF.Tanh, scale=fv)
                nc.scalar.activation(out=t2[:, sl], in_=tsc[:, sl],
                                     func=AF.Square)

        # loss_g = 1 - t2_g (DVE tensor_scalar, 4x mode); gates only the
        # per-chunk GL/P products, not the S-sum.
        ls3 = loss.tile([128, L3], BF, tag="L3")
        ls2 = loss.tile([128, L3], BF, tag="L2")
        ls1 = loss.tile([128, L3], BF, tag="L1")
        nls = ((t2_3, ls3), (t2_2, ls2), (t2_1, ls1))

        def neg_emit(sl):
            for t2, ls in nls:
                nc.vector.tensor_scalar(out=ls[:, sl], in0=t2[:, sl],
                                        scalar1=-1.0, scalar2=1.0,
                                        op0=ALU.mult, op1=ALU.add)

        neg_emit(SL0)

        # S-sum terms: PSUM = 7 - sum(t2 terms) accumulated on the PE
        terms = [(t2_1, 2), (t2_1, 3), (t2_2, 0), (t2_2, 2), (t2_3, 0), (t2_3, 3)]
        W_of = {}
        G_of = {}

        def emit_front(c):
            """msum (PE) -> G (DVE recip + ACT cast) -> P/GL/W (DVE).
            The recip is emitted before the muls: msum is a ring of one,
            so the next chunk's S-sum waits on it."""
            lo = c * CW
            msum_ps = pm.tile([128, CW], F32, tag="ms", name=f"msum{c}")
            for h in range(2):
                base = lo + h * 512
                sub = msum_ps[:, h * 512:(h + 1) * 512]
                nc.tensor.matmul(out=sub, lhsT=nid, rhs=q512,
                                 start=True, stop=False)
                for t, (arr, off) in enumerate(terms):
                    nc.tensor.matmul(
                        out=sub, lhsT=nid,
                        rhs=arr[:, base + off:base + off + 512],
                        start=False, stop=(t == 5),
                    )
            # G cast runs on the DVE: on ACT it would queue behind the
            # whole loss chain and stall every chunk's W muls by ~10us.
            G = ck.tile([128, CW], BF, tag="G4", name=f"G_{c}")
            G32 = ck.tile([128, CW], F32, tag="G32", name=f"G32_{c}")
            nc.vector.reciprocal_approx_fast(out=G32, in_=msum_ps)
            nc.vector.tensor_copy(out=G, in_=G32)
            G_of[c] = G

            Pc0 = ck.tile([128, CW], BF, tag="P0", name=f"P0_{c}")
            Pc1 = ck.tile([128, CW], BF, tag="P1", name=f"P1_{c}")
            Pc2 = ck.tile([128, CW], BF, tag="P2", name=f"P2_{c}")
            nc.vector.tensor_mul(out=Pc0, in0=ls3[:, lo:lo + CW],
                                 in1=xp[:, lo:lo + CW])
            nc.vector.tensor_mul(out=Pc1, in0=ls2[:, lo:lo + CW],
                                 in1=xp[:, lo + 1:lo + 1 + CW])
            nc.vector.tensor_mul(out=Pc2, in0=ls1[:, lo + 2:lo + 2 + CW],
                                 in1=xp[:, lo + 2:lo + 2 + CW])

            GL1 = ck.tile([128, CW], BF, tag="GL1", name=f"GL1_{c}")
            GL2 = ck.tile([128, CW], BF, tag="GL2", name=f"GL2_{c}")
            GL3 = ck.tile([128, CW], BF, tag="GL3", name=f"GL3_{c}")
            nc.vector.tensor_mul(out=GL1, in0=ls1[:, lo + 3:lo + 3 + CW], in1=G)
            nc.vector.tensor_mul(out=GL2, in0=ls2[:, lo + 2:lo + 2 + CW], in1=G)
            nc.vector.tensor_mul(out=GL3, in0=ls3[:, lo + 3:lo + 3 + CW], in1=G)

            W = [ck.tile([128, CW], BF, tag=f"W{k}", name=f"W{k}_{c}")
                 for k in range(KS)]
            nc.vector.tensor_mul(out=W[0], in0=G, in1=Pc0)
            nc.vector.tensor_mul(out=W[1], in0=G, in1=Pc1)
            nc.vector.tensor_mul(out=W[2], in0=G, in1=Pc2)
            nc.vector.tensor_mul(out=W[3], in0=G, in1=xp[:, lo + 3:lo + 3 + CW])
            nc.vector.tensor_mul(out=W[4], in0=GL1, in1=xp[:, lo + 4:lo + 4 + CW])
            nc.vector.tensor_mul(out=W[5], in0=GL2, in1=xp[:, lo + 5:lo + 5 + CW])
            nc.vector.tensor_mul(out=W[6], in0=GL3, in1=xp[:, lo + 6:lo + 6 + CW])
            W_of[c] = W

        def emit_back(c):
            """GEMM (PE, b0/b1 interleaved k-by-k so the two batches run on
            disjoint row-groups concurrently) -> obuf copies -> DMA."""
            lo = c * CW
            W = W_of[c]
            obuf = stp.tile([128, 2, CW], BF, tag="obuf", name=f"obuf_{c}")
            for sub_i in range(CW // 512):
                cs = slice(sub_i * 512, (sub_i + 1) * 512)
                acc = [ppa.tile([128, 512], F32, tag="acc",
                                name=f"acc_{c}_{b}_{sub_i}") for b in range(2)]
                for k in range(KS):
                    for b in range(2):
                        prow = slice(64 * b, 64 * (b + 1))
                        nc.tensor.matmul(
                            out=acc[b][:, :],
                            lhsT=fckf[prow, k * 128:(k + 1) * 128],
                            rhs=W[k][prow, cs],
                            start=(k == 0), stop=(k == KS - 1),
                        )
                for b in range(2):
                    # last chunk: split copies DVE/ACT so the tail runs
                    # them in parallel (DVE is already drained by then)
                    if c == NCH - 1 and sub_i == 1:
                        nc.vector.tensor_copy(out=obuf[:, b, cs], in_=acc[b])
                    else:
                        nc.scalar.copy(out=obuf[:, b, cs], in_=acc[b])
                if c == NCH - 1:
                    # last chunk: DMA each 512-sub as it lands
                    nc.sync.dma_start(out=out_d[:, c, :, cs],
                                      in_=obuf[:, :, cs])
            if c != NCH - 1:
                nc.sync.dma_start(out=out_d[:, c, :, :], in_=obuf)

        # Emission order: chunk 0 is covered by slice 0, chunk 1 by
        # slices 0-1, chunks 2/3 by the B slice; negates for each slice
        # land just before the first front that needs them.
        emit_front(0)
        neg_emit(SL1)
        emit_front(1)
        emit_back(0)
        neg_emit(SL_B)
        emit_back(1)
        emit_front(2)
        emit_back(2)
        emit_front(3)
        # a few more keep-warm matmuls riding chunk 3's G tile: the PE
        # idles ~4us while the last W muls run, and a re-throttled HAM
        # would run the last GEMM at half clock.
        scr2 = ppa.tile([128, 512], F32, tag="acc", name="warmmm2")
        for i in range(8):
            nc.tensor.matmul(out=scr2, lhsT=q512[:, 0:128],
                             rhs=G_of[3][:, 0:512],
                             start=True, stop=True, skip_group_check=True)
        emit_back(3)


def _trim_sem_budget():
    """Append --max-sem-num to the walrus backend options: the NEFF
    postamble serially resets every allocated semaphore (~10us at the
    default 192); a smaller budget shortens it. No-op if flags are
    unavailable."""
    try:
        from concourse.compiler_utils import (
            get_compiler_flags, set_compiler_flags)
    except Exception:
        return
    flags = get_compiler_flags()
    if any("--max-sem-num=96" in fl for fl in flags):
        return
    out, patched = [], False
    for fl in flags:
        if fl.startswith("--internal-backend-options="):
            fl = fl + " --max-sem-num=96"
            patched = True
        out.append(fl)
    if not patched:
        out.append("--internal-backend-options=--max-sem-num=96")
    set_compiler_flags(out)


def build_nc():
    _trim_sem_budget()
    nc = bacc_mod.Bacc(None, target_bir_lowering=False)
    blobA_d = nc.dram_tensor("blobA", [64, 512 + 2 * ND], BF,
                             kind="ExternalInput")
    fv_d = nc.dram_tensor("fv", [128, 1], F32, kind="ExternalInput")
    blobB_d = nc.dram_tensor("blobB", [128, 128 + KS * 128 + NP], BF,
                             kind="ExternalInput")
    out_d = nc.dram_tensor("out", [128, NCH, 2, CW], BF, kind="ExternalOutput")
    with tile.TileContext(nc) as tc:
        kernel_body(tc, blobA_d, fv_d, blobB_d, out_d)
    nc.compile()
    return nc


def _interp_host(dp, n_out):
    # exact port of the reference's interp_linear on the last dim (fp64)
    n_in = dp.shape[-1]
    src = (np.arange(n_out, dtype=np.float64) + 0.5) * (n_in / n_out) - 0.5
    src = np.clip(src, 0.0, n_in - 1.0)
    lo = np.floor(src).astype(np.int64)
    hi = np.minimum(lo + 1, n_in - 1)
    w = src - lo
    return dp[..., lo] * (1.0 - w) + dp[..., hi] * w


def prep_inputs(deep, x, conv_w, conv_b, fc_w):
    deep = np.asarray(deep, np.float32)
    x = np.asarray(x, np.float32)
    conv_w = np.asarray(conv_w, np.float32)
    conv_b = np.asarray(conv_b, np.float32)
    fc_w = np.asarray(fc_w, np.float32)

    xpad = np.pad(x, ((0, 0), (0, 0), (PAD, PAD)), mode="reflect")
    xp_all = np.ascontiguousarray(xpad.reshape(NCORES, 128, NP)).astype(bf16)
    dp_all = np.ascontiguousarray(deep.reshape(NCORES, 32, ND))
    # phase-fused conv+interp weights: y[4j+r] = a_r*ys[j] + b_r*ys[j+s_r]
    a_ph = [0.625, 0.875, 0.875, 0.625]
    b_ph = [0.375, 0.125, 0.125, 0.375]
    cwT = conv_w.T  # (16, 64)
    cph = np.zeros((64, 4, 128), np.float32)
    for r in range(4):
        cph[0:16, r, 0:64] = a_ph[r] * cwT
        cph[16:32, r, 64:128] = a_ph[r] * cwT
        cph[32:48, r, 0:64] = b_ph[r] * cwT
        cph[48:64, r, 64:128] = b_ph[r] * cwT
    cph = np.ascontiguousarray(cph).astype(bf16)
    nid = (-np.eye(128)).astype(bf16)
    fc3 = fc_w.reshape(128, 64, KS)
    fck_half = np.transpose(fc3, (1, 2, 0)).copy()
    fck = np.ascontiguousarray(
        np.concatenate([fck_half, fck_half], axis=0)).astype(bf16)

    # exact per-channel variance of y on the host (fp64): f = GAMA/(var+EPS)
    xr = _interp_host(deep.astype(np.float64), N)          # (16, dc, N)
    s1 = xr.sum(axis=2)                                    # (16, dc)
    g2 = np.einsum('bdn,ben->bde', xr, xr)                 # (16, dc, dc)
    w64 = conv_w.astype(np.float64)
    cb64 = conv_b.astype(np.float64)
    sy = np.einsum('cd,bd->bc', w64, s1) + N * cb64[None, :]
    sy2 = (np.einsum('cd,bde,ce->bc', w64, g2, w64)
           + 2.0 * cb64[None, :] * np.einsum('cd,bd->bc', w64, s1)
           + N * cb64[None, :] ** 2)
    mean = sy / N
    var = (sy2 - N * mean ** 2) / (N - 1)
    f = (GAMA / (var + EPS)).astype(np.float32)            # (16, 64)
    f_all = f.reshape(NCORES, 128, 1)

    # blobA: cph flattened [64, 512] | dpq01 [64, 1024] | dpq23 [64, 1024]
    cph_flat = cph.reshape(64, 512)
    maps = []
    for ci in range(NCORES):
        dp2 = dp_all[ci]                                  # (32, ND) b0;b1
        dpm = np.concatenate([dp2[:, :1], dp2[:, :-1]], axis=1)   # dp[j-1]
        dpp = np.concatenate([dp2[:, 1:], dp2[:, -1:]], axis=1)   # dp[j+1]
        dpq01 = np.concatenate([dp2, dpm], axis=0).astype(bf16)   # [64, ND]
        dpq23 = np.concatenate([dp2, dpp], axis=0).astype(bf16)
        blobA = np.ascontiguousarray(
            np.concatenate([cph_flat, dpq01, dpq23], axis=1))
        blobB = np.ascontiguousarray(np.concatenate(
            [nid, fck.reshape(128, KS * 128), xp_all[ci]], axis=1))
        maps.append({"blobA": blobA, "fv": np.ascontiguousarray(f_all[ci]),
                     "blobB": blobB})
    return maps


def gather_out(results):
    out_full = np.empty((16, 128, N), np.float32)
    for ci in range(NCORES):
        o = np.asarray(results[ci]["out"], dtype=np.float32)
        o = np.transpose(o, (0, 2, 1, 3)).reshape(128, 2, N)
        out_full[2 * ci] = o[:, 0]
        out_full[2 * ci + 1] = o[:, 1]
    return out_full


_CACHED = {}


def _get_nc():
    if "nc" not in _CACHED:
        _CACHED["nc"] = build_nc()
    return _CACHED["nc"]


def kernel(deep, x, conv_w, conv_b, fc_w):
    in_maps = prep_inputs(deep, x, conv_w, conv_b, fc_w)
    nc = _get_nc()
    res = run_bass_kernel_spmd(nc, in_maps, core_ids=list(range(NCORES)))
    return gather_out(res.results)


# revision 28
# speedup vs baseline: 1.0081x; 1.0070x over previous
"""Trainium2 Bass kernel for nn_DeepConv1d (self-contained).

Math (per batch b):
  xr   = linear-interp(deep, 1024 -> 4096)           # commutes with 1x1 conv
  y    = conv_w @ xr + conv_b                        # == interp(conv_w @ deep + conv_b)
  xs   = GAMA*(y-mean)/(var_unbiased+EPS)            # per-channel over n
  loss_k[c,l] = sech^2(f*(y_pad[c,l+k]-y_pad[c,l+3]))  # k=0..6, reflect pad 3
  S    = sum_k loss_k ;  W_k = (loss_k/S)*x_pad[:,l+k]
  out[o,l] = sum_{c,k} fc_w[o, 7c+k] * W_k[c,l]

On-chip identities / structure:
  - interp(conv(.)) == conv(interp(.)); conv+interp fused as 4 phase
    matmuls against host-stacked [dp; dp_shifted] (y bias dropped: it
    cancels in the y-differences, and mean/var are computed exactly on
    the host, so f = GAMA/(var+EPS) arrives as a per-channel constant).
  - loss = sech^2 = 1 - tanh^2: ACT Tanh (scale=f) then ACT Square give
    t2_g = tanh^2 per gap g=|k-3| in {1,2,3}; a DVE tensor_scalar
    (4x mode) forms loss_g = 1 - t2_g for the W products.
  - S-sum runs on the PE from the t2 arrays directly: PSUM =
    7 - sum(t2 terms) via 6 accumulating matmuls with lhsT = -I plus a
    +7 constant pass, so no extra DVE work on the S path.
  - G = 1/S via DVE reciprocal_approx_fast (fp32) + ACT cast to bf16.
  - Final GEMM (7 taps, contract 64 per batch) interleaves the two
    batches k-by-k: their lhsT/rhs live on disjoint partition halves so
    the PE runs them concurrently on separate row-groups/PSUM banks.

Layout: 2 batches per core packed on 128 partitions (64 channels each).
"""
import contextlib

import numpy as np
import ml_dtypes

import concourse.bass as bass
import concourse.bacc as bacc_mod
import concourse.mybir as mybir
import concourse.tile as tile
from concourse.bass_utils import run_bass_kernel_spmd

bf16 = ml_dtypes.bfloat16
AF = mybir.ActivationFunctionType
ALU = mybir.AluOpType

KS = 7
PAD = 3
GAMA = 0.5
EPS = 1e-9
N = 4096
ND = 1024
NP = N + 2 * PAD       # 4102
L3 = N + PAD           # 4099: gap-array length
NCORES = 8
NCH = 4                # l-chunks
CW = N // NCH          # 1024

F32 = mybir.dt.float32
BF = mybir.dt.bfloat16

HH = 2052              # A-half width for dy/tanh/square slices
H0 = 1032              # first slice: covers chunk-0 S-sum + GL reads
NWARM = 46             # PE keep-warm dummy matmuls between conv and S-sums


def kernel_body(tc, blobA_d, fv_d, blobB_d, out_d):
    nc = tc.nc

    ctx = contextlib.ExitStack()
    with ctx:
        io = ctx.enter_context(tc.tile_pool(name="io", bufs=1))
        mid = ctx.enter_context(tc.tile_pool(name="mid", bufs=1))
        loss = ctx.enter_context(tc.tile_pool(name="loss", bufs=1))
        ck = ctx.enter_context(tc.tile_pool(name="ck", bufs=2))
        stp = ctx.enter_context(tc.tile_pool(name="stp", bufs=3))
        pp = ctx.enter_context(tc.tile_pool(name="pp", bufs=2, space="PSUM"))
        pm = ctx.enter_context(tc.tile_pool(name="pm", bufs=1, space="PSUM"))
        ppa = ctx.enter_context(tc.tile_pool(name="ppa", bufs=4, space="PSUM"))

        # ------------- input DMAs (conv-critical, small, first) --------
        # cph/dpq stay separate transfers: the first conv LDW/matmul waits
        # on their individual completion semaphores, so a merged blob
        # would delay conv to the blob's full arrival.
        cph = io.tile([64, 512], BF, tag="cph")      # 4 phases x 128, flat
        nc.sync.dma_start(out=cph, in_=blobA_d[:, 0:512])
        dpq01 = io.tile([64, ND], BF, tag="dpq01")
        nc.sync.dma_start(out=dpq01, in_=blobA_d[:, 512:512 + ND])
        dpq23 = io.tile([64, ND], BF, tag="dpq23")
        nc.sync.dma_start(out=dpq23, in_=blobA_d[:, 512 + ND:])
        fv = io.tile([128, 1], F32, tag="fv")
        nc.sync.dma_start(out=fv, in_=fv_d[:, :])
        # blobB [128, 128+896+4102]: nid | fck (7x128) | xp
        blobB = io.tile([128, 128 + KS * 128 + NP], BF, tag="blobB")
        nc.sync.dma_start(out=blobB, in_=blobB_d[:, :])
        nid = blobB[:, 0:128]
        fckf = blobB[:, 128:128 + KS * 128]          # [128, 7*128] flat
        xp = blobB[:, 128 + KS * 128:]               # [128, NP] reflect-padded x

        # warm the tanh table with a DMA-independent input
        wz = mid.tile([128, 1], F32, tag="wz")
        nc.vector.memset(wz, 0.0)
        warm = mid.tile([128, 1], F32, tag="warm")
        nc.scalar.activation(out=warm, in_=wz, func=AF.Tanh, scale=1.0)

        # (-I) @ (-7) = +7 per partition: same nid lhsT as the t2 terms,
        # so the S-sum accumulation never switches weights.
        q512 = io.tile([128, 512], BF, tag="q512")
        nc.vector.memset(q512, -7.0)

        # ------- conv+interp fused on the PE (phase-decomposed) ----------
        # y[4j+r] = a_r*ys[j] + b_r*ys[j+s_r]  (s=-1 for r<2, +1 for r>=2)
        # == one matmul per phase against host-stacked [dp; dp_shifted].
        # h-major order through a ring of 2 one-bank PSUM tiles, so the
        # DVE can interleave the four h0 halves (plus a 1-col touch-up of
        # each h1 tile) and start dy after a few short casts.
        ypad = mid.tile([128, NP], BF, tag="ypad")
        yph = {}
        for h in range(2):
            for r in range(4):
                ypr = pp.tile([128, 512], F32, tag="ys", name=f"yph{r}_{h}")
                dq = dpq01 if r < 2 else dpq23
                nc.tensor.matmul(
                    out=ypr,
                    lhsT=cph[:, r * 128:(r + 1) * 128],
                    rhs=dq[:, h * 512:(h + 1) * 512],
                    start=True, stop=True,
                )
                yph[(r, h)] = ypr

        # PE keep-warm: dummy matmuls into a scratch bank (output never
        # read). rhs slices of xp make them eligible only once the big
        # input DMA lands (~15us), so the scheduler cannot hoist them
        # before conv; back-to-back they bridge the PE-idle dy/tanh
        # window so the HAM stays at K=8/8 for the S-sums and GEMMs.
        scr = ppa.tile([128, 512], F32, tag="acc", name="warmmm")
        for i in range(NWARM):
            nc.tensor.matmul(out=scr, lhsT=q512[:, 0:128],
                             rhs=xp[:, (i % 7) * 512:(i % 7) * 512 + 512],
                             start=True, stop=True, skip_group_check=True)

        def interleave(r, h, j0, jw):
            dst = bass.AP(tensor=ypad.tensor,
                          offset=ypad.offset + PAD + r + 4 * (h * 512 + j0),
                          ap=[list(ypad.ap[0]), [4, jw]])
            nc.vector.tensor_copy(out=dst, in_=yph[(r, h)][:, j0:j0 + jw])

        # slice 0 casts: ypad[:, :1035) -> dy slice 0 -> tanh starts early
        JS = (H0 + 3 + 3) // 4  # 259: j-cols per phase covering ypad[0,1035)
        for r in range(4):
            interleave(r, 0, 0, JS)
        # reflect front edges: ypad[2-i] = ypad[4+i]
        for i in range(3):
            nc.vector.tensor_copy(out=ypad[:, 2 - i:3 - i],
                                  in_=ypad[:, 4 + i:5 + i])

        dy1 = loss.tile([128, L3], BF, tag="T1")
        dy2b = loss.tile([128, L3], BF, tag="T2")
        dy3 = loss.tile([128, L3], BF, tag="T3")
        SL0, SL1 = slice(0, H0), slice(H0, HH)
        SL_B = slice(HH, L3)

        def dy_emit(sl):
            lo, hi = sl.start, sl.stop
            w = hi - lo
            nc.vector.tensor_sub(out=dy3[:, sl], in0=ypad[:, lo + 3:lo + 3 + w],
                                 in1=ypad[:, lo:hi])
            nc.vector.tensor_sub(out=dy2b[:, sl], in0=ypad[:, lo + 3:lo + 3 + w],
                                 in1=ypad[:, lo + 1:lo + 1 + w])
            nc.vector.tensor_sub(out=dy1[:, sl], in0=ypad[:, lo + 1:lo + 1 + w],
                                 in1=ypad[:, lo:hi])

        dy_emit(SL0)
        # rest of h0 + the h1 touch-up col -> dy slice 1
        for r in range(4):
            interleave(r, 0, JS, 512 - JS)
        for r in range(4):
            interleave(r, 1, 0, 1)   # col 2051+r: completes ypad[:, :2055)
        dy_emit(SL1)
        # B-half interleaves + tail edges + B-half dys
        for r in range(4):
            interleave(r, 1, 1, 511)
        for i in range(3):
            nc.vector.tensor_copy(out=ypad[:, N + 3 + i:N + 4 + i],
                                  in_=ypad[:, N + 1 - i:N + 2 - i])
        dy_emit(SL_B)

        # ------------- loss: t = tanh(f*dy); t2 = t^2 (ACT) -------------
        # t is a single scratch tile (all producers/consumers on ACT, so
        # reuse costs no cross-engine sync); t2 arrays feed the PE S-sum
        # and the DVE negates. Sliced so the chunk-0 S-sum and muls can
        # start after the first third of the chain.
        tsc = loss.tile([128, L3], BF, tag="TS")
        t2_3 = loss.tile([128, L3], BF, tag="Q3")
        t2_2 = loss.tile([128, L3], BF, tag="Q2")
        t2_1 = loss.tile([128, L3], BF, tag="Q1")
        gaps = ((dy3, t2_3), (dy2b, t2_2), (dy1, t2_1))
        for sl in (SL0, SL1, SL_B):
            for dy, t2 in gaps:
                nc.scalar.activation(out=tsc[:, sl], in_=dy[:, sl],
                                     func=AF.Tanh, scale=fv)
                nc.scalar.activation(out=t2[:, sl], in_=tsc[:, sl],
                                     func=AF.Square)

        # loss_g = 1 - t2_g (DVE tensor_scalar, 4x mode); gates only the
        # per-chunk GL/P products, not the S-sum.
        ls3 = loss.tile([128, L3], BF, tag="L3")
        ls2 = loss.tile([128, L3], BF, tag="L2")
        ls1 = loss.tile([128, L3], BF, tag="L1")
        nls = ((t2_3, ls3), (t2_2, ls2), (t2_1, ls1))

        def neg_emit(sl):
            for t2, ls in nls:
                nc.vector.tensor_scalar(out=ls[:, sl], in0=t2[:, sl],
                                        scalar1=-1.0, scalar2=1.0,
                                        op0=ALU.mult, op1=ALU.add)

        neg_emit(SL0)

        # S-sum terms: PSUM = 7 - sum(t2 terms) accumulated on the PE
        terms = [(t2_1, 2), (t2_1, 3), (t2_2, 0), (t2_2, 2), (t2_3, 0), (t2_3, 3)]
        W_of = {}
        G_of = {}

        def emit_front(c):
            """msum (PE) -> G (DVE recip + ACT cast) -> P/GL/W (DVE).
            The recip is emitted before the muls: msum is a ring of one,
            so the next chunk's S-sum waits on it."""
            lo = c * CW
            msum_ps = pm.tile([128, CW], F32, tag="ms", name=f"msum{c}")
            for h in range(2):
                base = lo + h * 512
                sub = msum_ps[:, h * 512:(h + 1) * 512]
                nc.tensor.matmul(out=sub, lhsT=nid, rhs=q512,
                                 start=True, stop=False)
                for t, (arr, off) in enumerate(terms):
                    nc.tensor.matmul(
                        out=sub, lhsT=nid,
                        rhs=arr[:, base + off:base + off + 512],
                        start=False, stop=(t == 5),
                    )
            # G cast runs on the DVE: on ACT it would queue behind the
            # whole loss chain and stall every chunk's W muls by ~10us.
            G = ck.tile([128, CW], BF, tag="G4", name=f"G_{c}")
            G32 = ck.tile([128, CW], F32, tag="G32", name=f"G32_{c}")
            nc.vector.reciprocal_approx_fast(out=G32, in_=msum_ps)
            nc.vector.tensor_copy(out=G, in_=G32)
            G_of[c] = G

            Pc0 = ck.tile([128, CW], BF, tag="P0", name=f"P0_{c}")
            Pc1 = ck.tile([128, CW], BF, tag="P1", name=f"P1_{c}")
            Pc2 = ck.tile([128, CW], BF, tag="P2", name=f"P2_{c}")
            nc.vector.tensor_mul(out=Pc0, in0=ls3[:, lo:lo + CW],
                                 in1=xp[:, lo:lo + CW])
            nc.vector.tensor_mul(out=Pc1, in0=ls2[:, lo:lo + CW],
                                 in1=xp[:, lo + 1:lo + 1 + CW])
            nc.vector.tensor_mul(out=Pc2, in0=ls1[:, lo + 2:lo + 2 + CW],
                                 in1=xp[:, lo + 2:lo + 2 + CW])

            GL1 = ck.tile([128, CW], BF, tag="GL1", name=f"GL1_{c}")
            GL2 = ck.tile([128, CW], BF, tag="GL2", name=f"GL2_{c}")
            GL3 = ck.tile([128, CW], BF, tag="GL3", name=f"GL3_{c}")
            nc.vector.tensor_mul(out=GL1, in0=ls1[:, lo + 3:lo + 3 + CW], in1=G)
            nc.vector.tensor_mul(out=GL2, in0=ls2[:, lo + 2:lo + 2 + CW], in1=G)
            nc.vector.tensor_mul(out=GL3, in0=ls3[:, lo + 3:lo + 3 + CW], in1=G)

            W = [ck.tile([128, CW], BF, tag=f"W{k}", name=f"W{k}_{c}")
                 for k in range(KS)]
            nc.vector.tensor_mul(out=W[0], in0=G, in1=Pc0)
            nc.vector.tensor_mul(out=W[1], in0=G, in1=Pc1)
            nc.vector.tensor_mul(out=W[2], in0=G, in1=Pc2)
            nc.vector.tensor_mul(out=W[3], in0=G, in1=xp[:, lo + 3:lo + 3 + CW])
            nc.vector.tensor_mul(out=W[4], in0=GL1, in1=xp[:, lo + 4:lo + 4 + CW])
            nc.vector.tensor_mul(out=W[5], in0=GL2, in1=xp[:, lo + 5:lo + 5 + CW])
            nc.vector.tensor_mul(out=W[6], in0=GL3, in1=xp[:, lo + 6:lo + 6 + CW])
            W_of[c] = W

        def emit_back(c):
            """GEMM (PE, b0/b1 interleaved k-by-k so the two batches run on
            disjoint row-groups concurrently) -> obuf copies -> DMA."""
            lo = c * CW
            W = W_of[c]
            obuf = stp.tile([128, 2, CW], BF, tag="obuf", name=f"obuf_{c}")
            for sub_i in range(CW // 512):
                cs = slice(sub_i * 512, (sub_i + 1) * 512)
                acc = [ppa.tile([128, 512], F32, tag="acc",
                                name=f"acc_{c}_{b}_{sub_i}") for b in range(2)]
                for k in range(KS):
                    for b in range(2):
                        prow = slice(64 * b, 64 * (b + 1))
                        nc.tensor.matmul(
                            out=acc[b][:, :],
                            lhsT=fckf[prow, k * 128:(k + 1) * 128],
                            rhs=W[k][prow, cs],
                            start=(k == 0), stop=(k == KS - 1),
                        )
                for b in range(2):
                    # last chunk: split copies DVE/ACT so the tail runs
                    # them in parallel (DVE is already drained by then)
                    if c == NCH - 1 and sub_i == 1:
                        nc.vector.tensor_copy(out=obuf[:, b, cs], in_=acc[b])
                    else:
                        nc.scalar.copy(out=obuf[:, b, cs], in_=acc[b])
                if c == NCH - 1:
                    # last chunk: DMA each 512-sub as it lands
                    nc.sync.dma_start(out=out_d[:, c, :, cs],
                                      in_=obuf[:, :, cs])
            if c != NCH - 1:
                nc.sync.dma_start(out=out_d[:, c, :, :], in_=obuf)

        # Emission order: chunk 0 is covered by slice 0, chunk 1 by
        # slices 0-1, chunks 2/3 by the B slice; negates for each slice
        # land just before the first front that needs them.
        emit_front(0)
        neg_emit(SL1)
        emit_front(1)
        emit_back(0)
        neg_emit(SL_B)
        emit_back(1)
        emit_front(2)
        emit_back(2)
        emit_front(3)
        # a few more keep-warm matmuls riding chunk 3's G tile: the PE
        # idles ~4us while the last W muls run, and a re-throttled HAM
        # would run the last GEMM at half clock.
        scr2 = ppa.tile([128, 512], F32, tag="acc", name="warmmm2")
        for i in range(8):
            nc.tensor.matmul(out=scr2, lhsT=q512[:, 0:128],
                             rhs=G_of[3][:, 0:512],
                             start=True, stop=True, skip_group_check=True)
        emit_back(3)


def _trim_sem_budget():
    """Shrink the DMA-queue semaphore pool (driver-level flags): the NEFF
    postamble serially resets every allocated semaphore -- ~256 of them
    (16 queues x 16 sems) at default settings, ~8us of pure epilogue.
    No-op if flags are unavailable."""
    try:
        from concourse.compiler_utils import (
            get_compiler_flags, set_compiler_flags)
    except Exception:
        return
    extra = ["--internal-num-semaphores-per-dma-queue=4",
             "--internal-num-hardware-queues-per-compiler-queue=8"]
    flags = get_compiler_flags()
    if extra[0] in flags:
        return
    set_compiler_flags(flags + extra)


def build_nc():
    _trim_sem_budget()
    nc = bacc_mod.Bacc(None, target_bir_lowering=False)
    blobA_d = nc.dram_tensor("blobA", [64, 512 + 2 * ND], BF,
                             kind="ExternalInput")
    fv_d = nc.dram_tensor("fv", [128, 1], F32, kind="ExternalInput")
    blobB_d = nc.dram_tensor("blobB", [128, 128 + KS * 128 + NP], BF,
                             kind="ExternalInput")
    out_d = nc.dram_tensor("out", [128, NCH, 2, CW], BF, kind="ExternalOutput")
    with tile.TileContext(nc) as tc:
        kernel_body(tc, blobA_d, fv_d, blobB_d, out_d)
    nc.compile()
    return nc


def _interp_host(dp, n_out):
    # exact port of the reference's interp_linear on the last dim (fp64)
    n_in = dp.shape[-1]
    src = (np.arange(n_out, dtype=np.float64) + 0.5) * (n_in / n_out) - 0.5
    src = np.clip(src, 0.0, n_in - 1.0)
    lo = np.floor(src).astype(np.int64)
    hi = np.minimum(lo + 1, n_in - 1)
    w = src - lo
    return dp[..., lo] * (1.0 - w) + dp[..., hi] * w


def prep_inputs(deep, x, conv_w, conv_b, fc_w):
    deep = np.asarray(deep, np.float32)
    x = np.asarray(x, np.float32)
    conv_w = np.asarray(conv_w, np.float32)
    conv_b = np.asarray(conv_b, np.float32)
    fc_w = np.asarray(fc_w, np.float32)

    xpad = np.pad(x, ((0, 0), (0, 0), (PAD, PAD)), mode="reflect")
    xp_all = np.ascontiguousarray(xpad.reshape(NCORES, 128, NP)).astype(bf16)
    dp_all = np.ascontiguousarray(deep.reshape(NCORES, 32, ND))
    # phase-fused conv+interp weights: y[4j+r] = a_r*ys[j] + b_r*ys[j+s_r]
    a_ph = [0.625, 0.875, 0.875, 0.625]
    b_ph = [0.375, 0.125, 0.125, 0.375]
    cwT = conv_w.T  # (16, 64)
    cph = np.zeros((64, 4, 128), np.float32)
    for r in range(4):
        cph[0:16, r, 0:64] = a_ph[r] * cwT
        cph[16:32, r, 64:128] = a_ph[r] * cwT
        cph[32:48, r, 0:64] = b_ph[r] * cwT
        cph[48:64, r, 64:128] = b_ph[r] * cwT
    cph = np.ascontiguousarray(cph).astype(bf16)
    nid = (-np.eye(128)).astype(bf16)
    fc3 = fc_w.reshape(128, 64, KS)
    fck_half = np.transpose(fc3, (1, 2, 0)).copy()
    fck = np.ascontiguousarray(
        np.concatenate([fck_half, fck_half], axis=0)).astype(bf16)

    # exact per-channel variance of y on the host (fp64): f = GAMA/(var+EPS)
    xr = _interp_host(deep.astype(np.float64), N)          # (16, dc, N)
    s1 = xr.sum(axis=2)                                    # (16, dc)
    g2 = np.einsum('bdn,ben->bde', xr, xr)                 # (16, dc, dc)
    w64 = conv_w.astype(np.float64)
    cb64 = conv_b.astype(np.float64)
    sy = np.einsum('cd,bd->bc', w64, s1) + N * cb64[None, :]
    sy2 = (np.einsum('cd,bde,ce->bc', w64, g2, w64)
           + 2.0 * cb64[None, :] * np.einsum('cd,bd->bc', w64, s1)
           + N * cb64[None, :] ** 2)
    mean = sy / N
    var = (sy2 - N * mean ** 2) / (N - 1)
    f = (GAMA / (var + EPS)).astype(np.float32)            # (16, 64)
    f_all = f.reshape(NCORES, 128, 1)

    # blobA: cph flattened [64, 512] | dpq01 [64, 1024] | dpq23 [64, 1024]
    cph_flat = cph.reshape(64, 512)
    maps = []
    for ci in range(NCORES):
        dp2 = dp_all[ci]                                  # (32, ND) b0;b1
        dpm = np.concatenate([dp2[:, :1], dp2[:, :-1]], axis=1)   # dp[j-1]
        dpp = np.concatenate([dp2[:, 1:], dp2[:, -1:]], axis=1)   # dp[j+1]
        dpq01 = np.concatenate([dp2, dpm], axis=0).astype(bf16)   # [64, ND]
        dpq23 = np.concatenate([dp2, dpp], axis=0).astype(bf16)
        blobA = np.ascontiguousarray(
            np.concatenate([cph_flat, dpq01, dpq23], axis=1))
        blobB = np.ascontiguousarray(np.concatenate(
            [nid, fck.reshape(128, KS * 128), xp_all[ci]], axis=1))
        maps.append({"blobA": blobA, "fv": np.ascontiguousarray(f_all[ci]),
                     "blobB": blobB})
    return maps


def gather_out(results):
    out_full = np.empty((16, 128, N), np.float32)
    for ci in range(NCORES):
        o = np.asarray(results[ci]["out"], dtype=np.float32)
        o = np.transpose(o, (0, 2, 1, 3)).reshape(128, 2, N)
        out_full[2 * ci] = o[:, 0]
        out_full[2 * ci + 1] = o[:, 1]
    return out_full


_CACHED = {}


def _get_nc():
    if "nc" not in _CACHED:
        _CACHED["nc"] = build_nc()
    return _CACHED["nc"]


def kernel(deep, x, conv_w, conv_b, fc_w):
    in_maps = prep_inputs(deep, x, conv_w, conv_b, fc_w)
    nc = _get_nc()
    res = run_bass_kernel_spmd(nc, in_maps, core_ids=list(range(NCORES)))
    return gather_out(res.results)


# revision 32
# speedup vs baseline: 1.0340x; 1.0258x over previous
"""Trainium2 Bass kernel for nn_DeepConv1d (self-contained).

Math (per batch b):
  xr   = linear-interp(deep, 1024 -> 4096)           # commutes with 1x1 conv
  y    = conv_w @ xr + conv_b                        # == interp(conv_w @ deep + conv_b)
  xs   = GAMA*(y-mean)/(var_unbiased+EPS)            # per-channel over n
  loss_k[c,l] = sech^2(f*(y_pad[c,l+k]-y_pad[c,l+3]))  # k=0..6, reflect pad 3
  S    = sum_k loss_k ;  W_k = (loss_k/S)*x_pad[:,l+k]
  out[o,l] = sum_{c,k} fc_w[o, 7c+k] * W_k[c,l]

On-chip identities / structure:
  - interp(conv(.)) == conv(interp(.)); conv+interp fused as 4 phase
    matmuls against host-stacked [dp; dp_shifted] (y bias dropped: it
    cancels in the y-differences, and mean/var are computed exactly on
    the host, so f = GAMA/(var+EPS) arrives as a per-channel constant).
  - loss = sech^2 = 1 - tanh^2: ACT Tanh (scale=f) then ACT Square give
    t2_g = tanh^2 per gap g=|k-3| in {1,2,3}; a DVE tensor_scalar
    (4x mode) forms loss_g = 1 - t2_g for the W products.
  - S-sum runs on the PE from the t2 arrays directly: PSUM =
    7 - sum(t2 terms) via 6 accumulating matmuls with lhsT = -I plus a
    +7 constant pass, so no extra DVE work on the S path.
  - G = 1/S via DVE reciprocal_approx_fast (fp32) + ACT cast to bf16.
  - Final GEMM (7 taps, contract 64 per batch) interleaves the two
    batches k-by-k: their lhsT/rhs live on disjoint partition halves so
    the PE runs them concurrently on separate row-groups/PSUM banks.

Layout: 2 batches per core packed on 128 partitions (64 channels each).
"""
import contextlib

import numpy as np
import ml_dtypes

import concourse.bass as bass
import concourse.bacc as bacc_mod
import concourse.mybir as mybir
import concourse.tile as tile
from concourse.bass_utils import run_bass_kernel_spmd

bf16 = ml_dtypes.bfloat16
AF = mybir.ActivationFunctionType
ALU = mybir.AluOpType

KS = 7
PAD = 3
GAMA = 0.5
EPS = 1e-9
N = 4096
ND = 1024
NP = N + 2 * PAD       # 4102
L3 = N + PAD           # 4099: gap-array length
NCORES = 8
NCH = 4                # l-chunks
CW = N // NCH          # 1024

F32 = mybir.dt.float32
BF = mybir.dt.bfloat16

HH = 2052              # A-half width for dy/tanh/square slices
H0 = 1032              # first slice: covers chunk-0 S-sum + GL reads
NWARM = 46             # PE keep-warm dummy matmuls between conv and S-sums


def kernel_body(tc, blobA_d, fv_d, blobB_d, out_d):
    nc = tc.nc

    ctx = contextlib.ExitStack()
    with ctx:
        io = ctx.enter_context(tc.tile_pool(name="io", bufs=1))
        mid = ctx.enter_context(tc.tile_pool(name="mid", bufs=1))
        loss = ctx.enter_context(tc.tile_pool(name="loss", bufs=1))
        ck = ctx.enter_context(tc.tile_pool(name="ck", bufs=2))
        stp = ctx.enter_context(tc.tile_pool(name="stp", bufs=3))
        pp = ctx.enter_context(tc.tile_pool(name="pp", bufs=2, space="PSUM"))
        pm = ctx.enter_context(tc.tile_pool(name="pm", bufs=1, space="PSUM"))
        ppa = ctx.enter_context(tc.tile_pool(name="ppa", bufs=4, space="PSUM"))

        # ------------- input DMAs (conv-critical, small, first) --------
        # cph/dpq stay separate transfers: the first conv LDW/matmul waits
        # on their individual completion semaphores, so a merged blob
        # would delay conv to the blob's full arrival.
        cph = io.tile([64, 512], BF, tag="cph")      # 4 phases x 128, flat
        dpq01 = io.tile([64, ND], BF, tag="dpq01")
        dpq23 = io.tile([64, ND], BF, tag="dpq23")
        # split the conv inputs so the first LDW/matmul can start on the
        # first fragments' completion instead of a whole-tensor DMA
        nc.sync.dma_start(out=cph[:, 0:256], in_=blobA_d[:, 0:256])
        nc.sync.dma_start(out=dpq01[:, 0:512], in_=blobA_d[:, 512:1024])
        nc.sync.dma_start(out=dpq23[:, 0:512],
                          in_=blobA_d[:, 512 + ND:1024 + ND])
        fv = io.tile([128, 1], F32, tag="fv")
        nc.sync.dma_start(out=fv, in_=fv_d[:, :])
        nc.sync.dma_start(out=cph[:, 256:512], in_=blobA_d[:, 256:512])
        nc.sync.dma_start(out=dpq01[:, 512:ND], in_=blobA_d[:, 1024:512 + ND])
        nc.sync.dma_start(out=dpq23[:, 512:ND], in_=blobA_d[:, 1024 + ND:])
        # blobB [128, 128+896+4102]: nid | fck (7x128) | xp
        blobB = io.tile([128, 128 + KS * 128 + NP], BF, tag="blobB")
        nc.sync.dma_start(out=blobB, in_=blobB_d[:, :])
        nid = blobB[:, 0:128]
        fckf = blobB[:, 128:128 + KS * 128]          # [128, 7*128] flat
        xp = blobB[:, 128 + KS * 128:]               # [128, NP] reflect-padded x

        # warm the tanh table with a DMA-independent input
        wz = mid.tile([128, 1], F32, tag="wz")
        nc.vector.memset(wz, 0.0)
        warm = mid.tile([128, 1], F32, tag="warm")
        nc.scalar.activation(out=warm, in_=wz, func=AF.Tanh, scale=1.0)

        # (-I) @ (-7) = +7 per partition: same nid lhsT as the t2 terms,
        # so the S-sum accumulation never switches weights.
        q512 = io.tile([128, 512], BF, tag="q512")
        nc.vector.memset(q512, -7.0)

        # ------- conv+interp fused on the PE (phase-decomposed) ----------
        # y[4j+r] = a_r*ys[j] + b_r*ys[j+s_r]  (s=-1 for r<2, +1 for r>=2)
        # == one matmul per phase against host-stacked [dp; dp_shifted].
        # h-major order through a ring of 2 one-bank PSUM tiles, so the
        # DVE can interleave the four h0 halves (plus a 1-col touch-up of
        # each h1 tile) and start dy after a few short casts.
        ypad = mid.tile([128, NP], BF, tag="ypad")
        yph = {}
        for h in range(2):
            for r in range(4):
                ypr = pp.tile([128, 512], F32, tag="ys", name=f"yph{r}_{h}")
                dq = dpq01 if r < 2 else dpq23
                nc.tensor.matmul(
                    out=ypr,
                    lhsT=cph[:, r * 128:(r + 1) * 128],
                    rhs=dq[:, h * 512:(h + 1) * 512],
                    start=True, stop=True,
                )
                yph[(r, h)] = ypr

        # PE keep-warm: dummy matmuls into a scratch bank (output never
        # read). rhs slices of xp make them eligible only once the big
        # input DMA lands (~15us), so the scheduler cannot hoist them
        # before conv; back-to-back they bridge the PE-idle dy/tanh
        # window so the HAM stays at K=8/8 for the S-sums and GEMMs.
        scr = ppa.tile([128, 512], F32, tag="acc", name="warmmm")
        for i in range(NWARM):
            nc.tensor.matmul(out=scr, lhsT=q512[:, 0:128],
                             rhs=xp[:, (i % 7) * 512:(i % 7) * 512 + 512],
                             start=True, stop=True, skip_group_check=True)

        def interleave(r, h, j0, jw):
            dst = bass.AP(tensor=ypad.tensor,
                          offset=ypad.offset + PAD + r + 4 * (h * 512 + j0),
                          ap=[list(ypad.ap[0]), [4, jw]])
            nc.vector.tensor_copy(out=dst, in_=yph[(r, h)][:, j0:j0 + jw])

        # slice 0 casts: ypad[:, :1035) -> dy slice 0 -> tanh starts early
        JS = (H0 + 3 + 3) // 4  # 259: j-cols per phase covering ypad[0,1035)
        for r in range(4):
            interleave(r, 0, 0, JS)
        # reflect front edges: ypad[2-i] = ypad[4+i]
        for i in range(3):
            nc.vector.tensor_copy(out=ypad[:, 2 - i:3 - i],
                                  in_=ypad[:, 4 + i:5 + i])

        dy1 = loss.tile([128, L3], BF, tag="T1")
        dy2b = loss.tile([128, L3], BF, tag="T2")
        dy3 = loss.tile([128, L3], BF, tag="T3")
        SL0, SL1 = slice(0, H0), slice(H0, HH)
        HB = 3080               # B split: chunk-2 S-sum needs t2 < 3079
        SLB1, SLB2 = slice(HH, HB), slice(HB, L3)
        SL_B = slice(HH, L3)

        def dy_emit(sl):
            lo, hi = sl.start, sl.stop
            w = hi - lo
            nc.vector.tensor_sub(out=dy3[:, sl], in0=ypad[:, lo + 3:lo + 3 + w],
                                 in1=ypad[:, lo:hi])
            nc.vector.tensor_sub(out=dy2b[:, sl], in0=ypad[:, lo + 3:lo + 3 + w],
                                 in1=ypad[:, lo + 1:lo + 1 + w])
            nc.vector.tensor_sub(out=dy1[:, sl], in0=ypad[:, lo + 1:lo + 1 + w],
                                 in1=ypad[:, lo:hi])

        dy_emit(SL0)
        # rest of h0 + the h1 touch-up col -> dy slice 1
        for r in range(4):
            interleave(r, 0, JS, 512 - JS)
        for r in range(4):
            interleave(r, 1, 0, 1)   # col 2051+r: completes ypad[:, :2055)
        dy_emit(SL1)
        # B-half interleaves + tail edges + B-half dys
        for r in range(4):
            interleave(r, 1, 1, 511)
        for i in range(3):
            nc.vector.tensor_copy(out=ypad[:, N + 3 + i:N + 4 + i],
                                  in_=ypad[:, N + 1 - i:N + 2 - i])
        dy_emit(SL_B)

        # ------------- loss: t = tanh(f*dy); t2 = t^2 (ACT) -------------
        # t is a single scratch tile (all producers/consumers on ACT, so
        # reuse costs no cross-engine sync); t2 arrays feed the PE S-sum
        # and the DVE negates. Sliced so the chunk-0 S-sum and muls can
        # start after the first third of the chain.
        tsc = loss.tile([128, L3], BF, tag="TS")
        t2_3 = loss.tile([128, L3], BF, tag="Q3")
        t2_2 = loss.tile([128, L3], BF, tag="Q2")
        t2_1 = loss.tile([128, L3], BF, tag="Q1")
        gaps = ((dy3, t2_3), (dy2b, t2_2), (dy1, t2_1))
        for sl in (SL0, SL1, SLB1, SLB2):
            for dy, t2 in gaps:
                nc.scalar.activation(out=tsc[:, sl], in_=dy[:, sl],
                                     func=AF.Tanh, scale=fv)
                nc.scalar.activation(out=t2[:, sl], in_=tsc[:, sl],
                                     func=AF.Square)

        # loss_g = 1 - t2_g (DVE tensor_scalar, 4x mode); gates only the
        # per-chunk GL/P products, not the S-sum.
        ls3 = loss.tile([128, L3], BF, tag="L3")
        ls2 = loss.tile([128, L3], BF, tag="L2")
        ls1 = loss.tile([128, L3], BF, tag="L1")
        nls = ((t2_3, ls3), (t2_2, ls2), (t2_1, ls1))

        def neg_emit(sl):
            for t2, ls in nls:
                nc.vector.tensor_scalar(out=ls[:, sl], in0=t2[:, sl],
                                        scalar1=-1.0, scalar2=1.0,
                                        op0=ALU.mult, op1=ALU.add)

        neg_emit(SL0)

        # S-sum terms: PSUM = 7 - sum(t2 terms) accumulated on the PE
        terms = [(t2_1, 2), (t2_1, 3), (t2_2, 0), (t2_2, 2), (t2_3, 0), (t2_3, 3)]
        W_of = {}
        G_of = {}

        def emit_front(c):
            """msum (PE) -> G (DVE recip + ACT cast) -> P/GL/W (DVE).
            The recip is emitted before the muls: msum is a ring of one,
            so the next chunk's S-sum waits on it."""
            lo = c * CW
            msum_ps = pm.tile([128, CW], F32, tag="ms", name=f"msum{c}")
            for h in range(2):
                base = lo + h * 512
                sub = msum_ps[:, h * 512:(h + 1) * 512]
                nc.tensor.matmul(out=sub, lhsT=nid, rhs=q512,
                                 start=True, stop=False)
                for t, (arr, off) in enumerate(terms):
                    nc.tensor.matmul(
                        out=sub, lhsT=nid,
                        rhs=arr[:, base + off:base + off + 512],
                        start=False, stop=(t == 5),
                    )
            # G cast runs on the DVE: on ACT it would queue behind the
            # whole loss chain and stall every chunk's W muls by ~10us.
            G = ck.tile([128, CW], BF, tag="G4", name=f"G_{c}")
            G32 = ck.tile([128, CW], F32, tag="G32", name=f"G32_{c}")
            nc.vector.reciprocal_approx_fast(out=G32, in_=msum_ps)
            nc.vector.tensor_copy(out=G, in_=G32)
            G_of[c] = G

            Pc0 = ck.tile([128, CW], BF, tag="P0", name=f"P0_{c}")
            Pc1 = ck.tile([128, CW], BF, tag="P1", name=f"P1_{c}")
            Pc2 = ck.tile([128, CW], BF, tag="P2", name=f"P2_{c}")
            nc.vector.tensor_mul(out=Pc0, in0=ls3[:, lo:lo + CW],
                                 in1=xp[:, lo:lo + CW])
            nc.vector.tensor_mul(out=Pc1, in0=ls2[:, lo:lo + CW],
                                 in1=xp[:, lo + 1:lo + 1 + CW])
            nc.vector.tensor_mul(out=Pc2, in0=ls1[:, lo + 2:lo + 2 + CW],
                                 in1=xp[:, lo + 2:lo + 2 + CW])

            GL1 = ck.tile([128, CW], BF, tag="GL1", name=f"GL1_{c}")
            GL2 = ck.tile([128, CW], BF, tag="GL2", name=f"GL2_{c}")
            GL3 = ck.tile([128, CW], BF, tag="GL3", name=f"GL3_{c}")
            nc.vector.tensor_mul(out=GL1, in0=ls1[:, lo + 3:lo + 3 + CW], in1=G)
            nc.vector.tensor_mul(out=GL2, in0=ls2[:, lo + 2:lo + 2 + CW], in1=G)
            nc.vector.tensor_mul(out=GL3, in0=ls3[:, lo + 3:lo + 3 + CW], in1=G)

            W = [ck.tile([128, CW], BF, tag=f"W{k}", name=f"W{k}_{c}")
                 for k in range(KS)]
            nc.vector.tensor_mul(out=W[0], in0=G, in1=Pc0)
            nc.vector.tensor_mul(out=W[1], in0=G, in1=Pc1)
            nc.vector.tensor_mul(out=W[2], in0=G, in1=Pc2)
            nc.vector.tensor_mul(out=W[3], in0=G, in1=xp[:, lo + 3:lo + 3 + CW])
            nc.vector.tensor_mul(out=W[4], in0=GL1, in1=xp[:, lo + 4:lo + 4 + CW])
            nc.vector.tensor_mul(out=W[5], in0=GL2, in1=xp[:, lo + 5:lo + 5 + CW])
            nc.vector.tensor_mul(out=W[6], in0=GL3, in1=xp[:, lo + 6:lo + 6 + CW])
            W_of[c] = W

        def emit_back(c):
            """GEMM (PE, b0/b1 interleaved k-by-k so the two batches run on
            disjoint row-groups concurrently) -> obuf copies -> DMA."""
            lo = c * CW
            W = W_of[c]
            obuf = stp.tile([128, 2, CW], BF, tag="obuf", name=f"obuf_{c}")
            for sub_i in range(CW // 512):
                cs = slice(sub_i * 512, (sub_i + 1) * 512)
                acc = [ppa.tile([128, 512], F32, tag="acc",
                                name=f"acc_{c}_{b}_{sub_i}") for b in range(2)]
                for k in range(KS):
                    for b in range(2):
                        prow = slice(64 * b, 64 * (b + 1))
                        nc.tensor.matmul(
                            out=acc[b][:, :],
                            lhsT=fckf[prow, k * 128:(k + 1) * 128],
                            rhs=W[k][prow, cs],
                            start=(k == 0), stop=(k == KS - 1),
                        )
                for b in range(2):
                    # last chunk: split copies DVE/ACT so the tail runs
                    # them in parallel (DVE is already drained by then)
                    if c == NCH - 1 and sub_i == 1:
                        nc.vector.tensor_copy(out=obuf[:, b, cs], in_=acc[b])
                    else:
                        nc.scalar.copy(out=obuf[:, b, cs], in_=acc[b])
                if c == NCH - 1:
                    # last chunk: DMA each 512-sub as it lands
                    nc.sync.dma_start(out=out_d[:, c, :, cs],
                                      in_=obuf[:, :, cs])
            if c != NCH - 1:
                nc.sync.dma_start(out=out_d[:, c, :, :], in_=obuf)

        # Emission order: chunk 0 is covered by slice 0, chunk 1 by
        # slices 0-1, chunks 2/3 by the B slice; negates for each slice
        # land just before the first front that needs them.
        emit_front(0)
        neg_emit(SL1)
        emit_front(1)
        emit_back(0)
        neg_emit(SLB1)
        emit_back(1)
        emit_front(2)
        neg_emit(SLB2)
        emit_back(2)
        emit_front(3)
        # a few more keep-warm matmuls riding chunk 3's G tile: the PE
        # idles ~4us while the last W muls run, and a re-throttled HAM
        # would run the last GEMM at half clock.
        scr2 = ppa.tile([128, 512], F32, tag="acc", name="warmmm2")
        for i in range(8):
            nc.tensor.matmul(out=scr2, lhsT=q512[:, 0:128],
                             rhs=G_of[3][:, 0:512],
                             start=True, stop=True, skip_group_check=True)
        emit_back(3)


def _trim_sem_budget():
    """Shrink the DMA-queue semaphore pool (driver-level flags): the NEFF
    postamble serially resets every allocated semaphore -- ~256 of them
    (16 queues x 16 sems) at default settings, ~8us of pure epilogue.
    No-op if flags are unavailable."""
    try:
        from concourse.compiler_utils import (
            get_compiler_flags, set_compiler_flags)
    except Exception:
        return
    extra = ["--internal-num-semaphores-per-dma-queue=4",
             "--internal-num-hardware-queues-per-compiler-queue=8"]
    flags = get_compiler_flags()
    if extra[0] in flags:
        return
    set_compiler_flags(flags + extra)


def build_nc():
    _trim_sem_budget()
    nc = bacc_mod.Bacc(None, target_bir_lowering=False)
    blobA_d = nc.dram_tensor("blobA", [64, 512 + 2 * ND], BF,
                             kind="ExternalInput")
    fv_d = nc.dram_tensor("fv", [128, 1], F32, kind="ExternalInput")
    blobB_d = nc.dram_tensor("blobB", [128, 128 + KS * 128 + NP], BF,
                             kind="ExternalInput")
    out_d = nc.dram_tensor("out", [128, NCH, 2, CW], BF, kind="ExternalOutput")
    with tile.TileContext(nc) as tc:
        kernel_body(tc, blobA_d, fv_d, blobB_d, out_d)
    nc.compile()
    return nc


def _interp_host(dp, n_out):
    # exact port of the reference's interp_linear on the last dim (fp64)
    n_in = dp.shape[-1]
    src = (np.arange(n_out, dtype=np.float64) + 0.5) * (n_in / n_out) - 0.5
    src = np.clip(src, 0.0, n_in - 1.0)
    lo = np.floor(src).astype(np.int64)
    hi = np.minimum(lo + 1, n_in - 1)
    w = src - lo
    return dp[..., lo] * (1.0 - w) + dp[..., hi] * w


def prep_inputs(deep, x, conv_w, conv_b, fc_w):
    deep = np.asarray(deep, np.float32)
    x = np.asarray(x, np.float32)
    conv_w = np.asarray(conv_w, np.float32)
    conv_b = np.asarray(conv_b, np.float32)
    fc_w = np.asarray(fc_w, np.float32)

    xpad = np.pad(x, ((0, 0), (0, 0), (PAD, PAD)), mode="reflect")
    xp_all = np.ascontiguousarray(xpad.reshape(NCORES, 128, NP)).astype(bf16)
    dp_all = np.ascontiguousarray(deep.reshape(NCORES, 32, ND))
    # phase-fused conv+interp weights: y[4j+r] = a_r*ys[j] + b_r*ys[j+s_r]
    a_ph = [0.625, 0.875, 0.875, 0.625]
    b_ph = [0.375, 0.125, 0.125, 0.375]
    cwT = conv_w.T  # (16, 64)
    cph = np.zeros((64, 4, 128), np.float32)
    for r in range(4):
        cph[0:16, r, 0:64] = a_ph[r] * cwT
        cph[16:32, r, 64:128] = a_ph[r] * cwT
        cph[32:48, r, 0:64] = b_ph[r] * cwT
        cph[48:64, r, 64:128] = b_ph[r] * cwT
    cph = np.ascontiguousarray(cph).astype(bf16)
    nid = (-np.eye(128)).astype(bf16)
    fc3 = fc_w.reshape(128, 64, KS)
    fck_half = np.transpose(fc3, (1, 2, 0)).copy()
    fck = np.ascontiguousarray(
        np.concatenate([fck_half, fck_half], axis=0)).astype(bf16)

    # exact per-channel variance of y on the host (fp64): f = GAMA/(var+EPS)
    xr = _interp_host(deep.astype(np.float64), N)          # (16, dc, N)
    s1 = xr.sum(axis=2)                                    # (16, dc)
    g2 = np.einsum('bdn,ben->bde', xr, xr)                 # (16, dc, dc)
    w64 = conv_w.astype(np.float64)
    cb64 = conv_b.astype(np.float64)
    sy = np.einsum('cd,bd->bc', w64, s1) + N * cb64[None, :]
    sy2 = (np.einsum('cd,bde,ce->bc', w64, g2, w64)
           + 2.0 * cb64[None, :] * np.einsum('cd,bd->bc', w64, s1)
           + N * cb64[None, :] ** 2)
    mean = sy / N
    var = (sy2 - N * mean ** 2) / (N - 1)
    f = (GAMA / (var + EPS)).astype(np.float32)            # (16, 64)
    f_all = f.reshape(NCORES, 128, 1)

    # blobA: cph flattened [64, 512] | dpq01 [64, 1024] | dpq23 [64, 1024]
    cph_flat = cph.reshape(64, 512)
    maps = []
    for ci in range(NCORES):
        dp2 = dp_all[ci]                                  # (32, ND) b0;b1
        dpm = np.concatenate([dp2[:, :1], dp2[:, :-1]], axis=1)   # dp[j-1]
        dpp = np.concatenate([dp2[:, 1:], dp2[:, -1:]], axis=1)   # dp[j+1]
        dpq01 = np.concatenate([dp2, dpm], axis=0).astype(bf16)   # [64, ND]
        dpq23 = np.concatenate([dp2, dpp], axis=0).astype(bf16)
        blobA = np.ascontiguousarray(
            np.concatenate([cph_flat, dpq01, dpq23], axis=1))
        blobB = np.ascontiguousarray(np.concatenate(
            [nid, fck.reshape(128, KS * 128), xp_all[ci]], axis=1))
        maps.append({"blobA": blobA, "fv": np.ascontiguousarray(f_all[ci]),
                     "blobB": blobB})
    return maps


def gather_out(results):
    out_full = np.empty((16, 128, N), np.float32)
    for ci in range(NCORES):
        o = np.asarray(results[ci]["out"], dtype=np.float32)
        o = np.transpose(o, (0, 2, 1, 3)).reshape(128, 2, N)
        out_full[2 * ci] = o[:, 0]
        out_full[2 * ci + 1] = o[:, 1]
    return out_full


_CACHED = {}


def _get_nc():
    if "nc" not in _CACHED:
        _CACHED["nc"] = build_nc()
    return _CACHED["nc"]


def kernel(deep, x, conv_w, conv_b, fc_w):
    in_maps = prep_inputs(deep, x, conv_w, conv_b, fc_w)
    nc = _get_nc()
    res = run_bass_kernel_spmd(nc, in_maps, core_ids=list(range(NCORES)))
    return gather_out(res.results)
